# revision 1
# baseline (speedup 1.0000x reference)
"""Trainium2 Bass kernel for nn_MultiHeadAttention (GQA + RoPE + causal softmax).

Problem (hardcoded): B=4, T=2048, C=2048, n_head=16, n_kv_head=4, head_dim=128,
fp32 in/out, rope base 10000, torch-Linear style projections (x @ W.T).

Sharding: 8 cores = (4 batches) x (2 head-halves, tensor parallel). Each core
handles one batch and 8 contiguous query heads (= 2 kv groups), full T=2048
sequence. O-projection is row-parallel: each core computes a partial [C, T]
output (contraction over its local 1024 y-channels); the host sums the two
partials per batch. No collectives.

Dataflow (single fused pass over x^T, all intermediates SBUF-resident):
  per 256-token block tb: K(tb), V(tb) projections run two blocks ahead of
  Q(tb) (so startup has PE work while wq streams in); RoPE via DMA half-swap,
  Pool muls and a DVE add. Flash-attention for query pair-block jj = tb-1 is
  emitted chunk-by-chunk INTERLEAVED between the QKV matmul groups so the
  Act-engine exp work hides under PE-dense projection matmuls.
  Per 128-key chunk: S^T[k,q] matmul (2 heads x 256q packed) -> additive
  causal mask (diagonal chunks only, jj-independent bf16 pattern) -> exp on
  Act -> P^T fp16 -> AV accumulate on PE (lagged one chunk). The softmax
  denominator is accumulated on DVE in fp16 (2-byte fast mode), reduced
  cross-partition by one M=1 ones-matmul per head-pair, then reciprocal (DVE)
  -> ones-broadcast matmul -> bs copy (Act) -> normalize mul (DVE) -> y bf16.
  Normalization is software-pipelined one head-pair behind the matmul stream
  (closure queue pumped at fixed points) so the PE never waits on it.
  Stage O runs rb-major, interleaved with attn(7) chunks; rb=3 (which reads
  attn(6/7) output columns) only starts after the deferred normalizes drain.

Matmul operands are bf16/fp16 (1 PE cycle/row, halves DMA + SBUF) except the
denominator path (f32r ones / fp16 acc). PSUM accumulation is f32 throughout.
"""

import sys
import math

sys.path.insert(0, "/opt/trn_rl_repo")

import numpy as np
import ml_dtypes

import concourse.bacc as bacc
import concourse.mybir as mybir
import concourse.tile as tile
from concourse.bass_utils import run_bass_kernel_spmd

F32 = mybir.dt.float32
F32R = mybir.dt.float32r
BF16 = mybir.dt.bfloat16
F16 = mybir.dt.float16
AF = mybir.ActivationFunctionType

B, T, C = 4, 2048, 2048
NH, NKV, HD = 16, 4, 128
ROPE_BASE = 10000.0
LH = 8                        # local q heads per core
LG = 2                        # local kv groups per core
NCC = C // 128                # 16 contraction chunks over C
NTB = T // 256                # 8 token blocks
MASK_NEG = -30000.0

DEBUG_DUMPS = False


def _build_nc(nrep=1):
    nc = bacc.Bacc(trn_type="TRN2", name="mha_tp")

    xT = nc.dram_tensor("xT", [C, T], BF16, kind="ExternalInput")
    wqT = nc.dram_tensor("wqT", [C, LH * HD], BF16, kind="ExternalInput")
    wkT = nc.dram_tensor("wkT", [C, LG * HD], BF16, kind="ExternalInput")
    wvT = nc.dram_tensor("wvT", [C, LG * HD], BF16, kind="ExternalInput")
    woT = nc.dram_tensor("woT", [LH * HD, C], BF16, kind="ExternalInput")
    cosk = nc.dram_tensor("cosk", [HD, T], BF16, kind="ExternalInput")
    sink = nc.dram_tensor("sink", [HD, T], BF16, kind="ExternalInput")
    maskadd = nc.dram_tensor("maskadd", [128, 768], BF16, kind="ExternalInput")
    ones_d = nc.dram_tensor("ones_d", [128, 128], F32R, kind="ExternalInput")
    outT = nc.dram_tensor("outT", [C, T], F32, kind="ExternalOutput")
    if DEBUG_DUMPS:
        dbg_k = nc.dram_tensor("dbg_k", [128, LG, T], BF16, kind="ExternalOutput")
        dbg_v = nc.dram_tensor("dbg_v", [128, T // 128, LG * HD], F16, kind="ExternalOutput")
        dbg_q = nc.dram_tensor("dbg_q", [128, LH, 256], BF16, kind="ExternalOutput")
        dbg_y = nc.dram_tensor("dbg_y", [128, LH, T], BF16, kind="ExternalOutput")
        dbg_den = nc.dram_tensor("dbg_den", [128, 512], F32, kind="ExternalOutput")

    with tile.TileContext(nc) as tc:
        with tc.tile_pool(name="const", bufs=1) as constp:
            ones_s = constp.tile([128, 128], F32R)
            nc.scalar.dma_start(out=ones_s[:], in_=ones_d.ap())
            ones16_s = constp.tile([128, 128], F16)
            nc.scalar.copy(ones16_s[:], ones_s[:])

            for _rep in range(nrep):
                with tc.tile_pool(name="res", bufs=1) as resp, \
                     tc.tile_pool(name="wres", bufs=1) as wresp:
                    # ---- initial loads, ordered so tb0's K can start ASAP ----
                    kT_s = resp.tile([128, LG, T], BF16)     # [d, g, t]
                    v_s = resp.tile([128, T // 128, LG * HD], F16)  # [t%128, tc, (g,vd)]
                    y_s = resp.tile([128, LH, T], BF16)      # [vd, h, t]

                    # issue order = DMA service order: small/urgent first
                    wk_s = wresp.tile([128, NCC, LG * HD], BF16, tag="wk")
                    nc.gpsimd.dma_start(
                        out=wk_s[:, :, 0:HD],
                        in_=wkT.ap()[:, 0:HD].rearrange("(c p) k -> p c k", p=128),
                    )
                    nc.gpsimd.dma_start(
                        out=wk_s[:, :, HD:2 * HD],
                        in_=wkT.ap()[:, HD:2 * HD].rearrange(
                            "(c p) k -> p c k", p=128
                        ),
                    )
                    wv_s = wresp.tile([128, NCC, LG * HD], BF16, tag="wv")
                    nc.gpsimd.dma_start(
                        out=wv_s[:], in_=wvT.ap().rearrange("(c p) k -> p c k", p=128)
                    )
                    cos_s = resp.tile([HD, T], BF16)
                    sin_s = resp.tile([HD, T], BF16)
                    mask_s = resp.tile([128, 768], BF16)
                    wq_s = wresp.tile([128, NCC, LH * HD], BF16, tag="wq")
                    wo_s = wresp.tile([128, LH, C], BF16, tag="wo")

                    def load_big_weights():
                        for qh in range(2):
                            hsl = slice(qh * 512, (qh + 1) * 512)
                            nc.sync.dma_start(
                                out=wq_s[:, :, hsl],
                                in_=wqT.ap()[:, hsl].rearrange(
                                    "(c p) m -> p c m", p=128
                                ),
                            )


                    with tc.tile_pool(name="xt", bufs=4) as xtp, \
                         tc.tile_pool(name="qbuf", bufs=2) as qbufp, \
                         tc.tile_pool(name="rope", bufs=2) as ropep, \
                         tc.tile_pool(name="ptile", bufs=4) as ptp, \
                         tc.tile_pool(name="dena", bufs=2) as denp, \
                         tc.tile_pool(name="bsb", bufs=2) as bsbp, \
                         tc.tile_pool(name="small", bufs=2) as smallp, \
                         tc.tile_pool(name="spsum", bufs=2, space="PSUM") as sps, \
                         tc.tile_pool(name="opsum", bufs=2, space="PSUM") as ops, \
                         tc.tile_pool(name="dbpsum", bufs=1, space="PSUM") as dbp:

                        pending = []  # deferred normalize closures
                        pending_rec = {}

                        def pump(n=1):
                            for _ in range(n):
                                if pending:
                                    pending.pop(0)()

                        def rope_write(ps, out_ap, tsl):
                            """out = ps*cos + swap(ps)*sin (sign folded in sin).
                            Muls on Pool from an Act-copied SBUF staging tile so
                            the PSUM tile drains through a single Act read."""
                            t0 = ropep.tile([128, 256], F32, tag="r0")
                            nc.scalar.copy(t0[:], ps[:])
                            rot = ropep.tile([128, 256], F32, tag="rot")
                            nc.scalar.dma_start(out=rot[0:64, :], in_=t0[64:128, :])
                            nc.scalar.dma_start(out=rot[64:128, :], in_=t0[0:64, :])
                            t1 = ropep.tile([128, 256], F32, tag="r1")
                            nc.gpsimd.tensor_mul(t1[:], t0[:], cos_s[:, tsl])
                            nc.gpsimd.tensor_mul(rot[:], rot[:], sin_s[:, tsl])
                            nc.vector.tensor_add(out_ap, t1[:], rot[:])

                        def attn_gen(jj, qp):
                            """Yields once per score chunk and once per pair end."""
                            qp_flat = qp[:].rearrange("p h q -> p (h q)")
                            for g in range(LG):
                                for hp in range(2):
                                    hh = g * 4 + hp * 2
                                    nch = 2 * jj + 2
                                    den_acc = denp.tile([128, 512], F16, tag="dacc")
                                    po = ops.tile([128, 512], F32, tag="po")
                                    prev_pt = None
                                    for cc in range(nch):
                                        if cc < nch - 1:
                                            pss = sps.tile([128, 512], F32, tag="pss")
                                            nc.tensor.matmul(
                                                pss[:],
                                                kT_s[:, g, cc * 128:(cc + 1) * 128],
                                                qp_flat[:, hh * 256:(hh + 2) * 256],
                                                start=True,
                                                stop=True,
                                            )
                                            if cc == 2 * jj:
                                                nc.vector.tensor_add(
                                                    pss[:], pss[:], mask_s[:, 0:512],
                                                )
                                            pt = ptp.tile([128, 512], F16, tag="pt")
                                            nc.scalar.activation(pt[:], pss[:], AF.Exp)
                                            with nc.allow_low_precision(reason="dacc"):
                                                if cc == 0:
                                                    nc.vector.tensor_copy(
                                                        den_acc[:], pt[:]
                                                    )
                                                else:
                                                    nc.vector.tensor_add(
                                                        den_acc[:], den_acc[:], pt[:]
                                                    )
                                        else:
                                            # second diagonal chunk: queries
                                            # 0..127 of each head are fully
                                            # masked -> compute only cols
                                            # 128..255 per head
                                            pss = sps.tile([128, 512], F32, tag="pss")
                                            nc.tensor.matmul(
                                                pss[:, 0:256],
                                                kT_s[:, g, cc * 128:(cc + 1) * 128],
                                                qp[:, hh:hh + 2, 128:256],
                                                start=True,
                                                stop=True,
                                            )
                                            nc.vector.tensor_add(
                                                pss[:, 0:256], pss[:, 0:256],
                                                mask_s[:, 512:768],
                                            )
                                            pt = ptp.tile([128, 256], F16, tag="ptd")
                                            nc.scalar.activation(
                                                pt[:], pss[:, 0:256], AF.Exp
                                            )
                                            da3 = den_acc[:].rearrange(
                                                "p (h q) -> p h q", h=2
                                            )
                                            pd3 = pt[:].rearrange(
                                                "p (h q) -> p h q", h=2
                                            )
                                            with nc.allow_low_precision(reason="dacc"):
                                                nc.vector.tensor_add(
                                                    da3[:, :, 128:256],
                                                    da3[:, :, 128:256],
                                                    pd3,
                                                )
                                        if prev_pt is not None:
                                            pcc = cc - 1
                                            nc.tensor.matmul(
                                                po[:],
                                                v_s[:, pcc, g * HD:(g + 1) * HD],
                                                prev_pt[:],
                                                start=(pcc == 0),
                                                stop=False,
                                            )
                                        prev_pt = pt
                                        if cc == 1:
                                            pump(1)
                                        yield
                                    po3 = po[:].rearrange("p (h q) -> p h q", h=2)
                                    nc.tensor.matmul(
                                        po3[:, :, 128:256],
                                        v_s[:, nch - 1, g * HD:(g + 1) * HD],
                                        prev_pt[:],
                                        start=False,
                                        stop=True,
                                    )
                                    pump(1)

                                    def norm_den(den_acc=den_acc, hh=hh, jj=jj):
                                        den = dbp.tile([1, 512], F32, tag="dbp")
                                        rec = smallp.tile([1, 512], F32R, tag="rec")
                                        nc.tensor.matmul(
                                            den[:], ones16_s[:, 0:1], den_acc[:],
                                            start=True, stop=True,
                                        )
                                        with nc.allow_low_precision(reason="recip"):
                                            nc.vector.reciprocal(rec[:], den[:])
                                        pending_rec[(hh, jj)] = rec

                                    def norm_mul(po=po, hh=hh, jj=jj):
                                        rec = pending_rec.pop((hh, jj))
                                        pb = dbp.tile([128, 512], F32, tag="dbp")
                                        nc.tensor.matmul(
                                            pb[:], ones_s[0:1, :], rec[:],
                                            start=True, stop=True,
                                        )
                                        bs = bsbp.tile([128, 512], F32, tag="bs")
                                        nc.vector.tensor_copy(bs[:], pb[:])
                                        qsl = slice(jj * 256, (jj + 1) * 256)
                                        nc.vector.tensor_mul(
                                            y_s[:, hh:hh + 2, qsl],
                                            po[:].rearrange("p (h q) -> p h q", h=2),
                                            bs[:].rearrange("p (h q) -> p h q", h=2),
                                        )

                                    pending.append(norm_den)
                                    pending.append(norm_mul)
                                    yield

                        def pull(gen, n):
                            if gen is None:
                                return gen
                            for _ in range(n):
                                try:
                                    next(gen)
                                except StopIteration:
                                    return None
                            return gen

                        qp_prev = None
                        gen = None
                        xts = {}

                        with tc.tile_pool(name="kvqps", bufs=3, space="PSUM") as kvqps:
                            def load_xt(tb):
                                tsl = slice(tb * 256, (tb + 1) * 256)
                                xt = xtp.tile([128, NCC, 256], BF16, tag="xt")
                                nc.sync.dma_start(
                                    out=xt[:],
                                    in_=xT.ap()[:, tsl].rearrange(
                                        "(c p) t -> p c t", p=128
                                    ),
                                )
                                xts[tb] = xt

                            def kv_block(tb, group_pull):
                                tsl = slice(tb * 256, (tb + 1) * 256)
                                xt = xts[tb]
                                for g in range(LG):
                                    psk = kvqps.tile([128, 256], F32, tag="kvq")
                                    for c in range(NCC):
                                        nc.tensor.matmul(
                                            psk[:],
                                            wk_s[:, c, g * HD:(g + 1) * HD],
                                            xt[:, c, :],
                                            start=(c == 0),
                                            stop=(c == NCC - 1),
                                        )
                                    rope_write(psk, kT_s[:, g, tsl], tsl)
                                    group_pull()
                                for ti in range(2):
                                    psv = kvqps.tile([128, LG * HD], F32, tag="kvq")
                                    for c in range(NCC):
                                        nc.tensor.matmul(
                                            psv[:],
                                            xt[:, c, ti * 128:(ti + 1) * 128],
                                            wv_s[:, c, :],
                                            start=(c == 0),
                                            stop=(c == NCC - 1),
                                        )
                                    nc.scalar.copy(v_s[:, tb * 2 + ti, :], psv[:])
                                    group_pull()

                            def noop_pull():
                                pass

                            # K/V run two blocks ahead of Q so startup has PE
                            # work while wq streams in
                            load_xt(0)
                            nc.sync.dma_start(out=cos_s[:], in_=cosk.ap())
                            nc.sync.dma_start(out=sin_s[:], in_=sink.ap())
                            load_xt(1)
                            nc.sync.dma_start(out=mask_s[:], in_=maskadd.ap())
                            load_xt(2)
                            load_big_weights()
                            kv_block(0, noop_pull)
                            kv_block(1, noop_pull)
                            kv_block(2, noop_pull)
                            for tb in range(NTB):
                                ngroups = 12 if tb + 3 < NTB else 8
                                steps = (8 * tb + 4) if tb > 0 else 0
                                k, rem = divmod(steps, ngroups)
                                gi = 0

                                def group_pull():
                                    nonlocal gen, gi
                                    gen = pull(gen, k + (1 if gi < rem else 0))
                                    gi += 1

                                # Q first: the xt slot WAR for tb+3 clears
                                # behind Q's matmuls, so KV(tb+3) never waits
                                if tb + 3 < NTB:
                                    load_xt(tb + 3)
                                if tb in (4, 5):
                                    osl = slice((tb - 4) * C // 2,
                                                (tb - 3) * C // 2)
                                    nc.scalar.dma_start(
                                        out=wo_s[:, :, osl],
                                        in_=woT.ap()[:, osl].rearrange(
                                            "(h p) c -> p h c", p=128
                                        ),
                                    )
                                tsl = slice(tb * 256, (tb + 1) * 256)
                                xt = xts[tb]
                                qp = qbufp.tile([128, LH, 256], BF16, tag="qp")
                                for hh in range(LH):
                                    psq = kvqps.tile([128, 256], F32, tag="kvq")
                                    for c in range(NCC):
                                        nc.tensor.matmul(
                                            psq[:],
                                            wq_s[:, c, hh * HD:(hh + 1) * HD],
                                            xt[:, c, :],
                                            start=(c == 0),
                                            stop=(c == NCC - 1),
                                        )
                                    rope_write(psq, qp[:, hh, :], tsl)
                                    group_pull()
                                del xts[tb]
                                if tb + 3 < NTB:
                                    kv_block(tb + 3, group_pull)
                                while gen is not None:
                                    gen = pull(gen, 4)
                                gen = attn_gen(tb, qp)
                                qp_prev = qp

                        if DEBUG_DUMPS:
                            while gen is not None:
                                gen = pull(gen, 16)
                            pump(len(pending))
                            nc.sync.dma_start(out=dbg_k.ap(), in_=kT_s[:])
                            nc.sync.dma_start(out=dbg_v.ap(), in_=v_s[:])
                            nc.sync.dma_start(out=dbg_q.ap(), in_=qp_prev[:])
                            nc.sync.dma_start(out=dbg_y.ap(), in_=y_s[:])

                        # ---- stage O (rb-major), interleaved with attn(7) ----
                        with tc.tile_pool(name="oout", bufs=3) as ooutp, \
                             tc.tile_pool(name="opsum2", bufs=2, space="PSUM") as ops2:
                            ocount = 0
                            for rb in range(T // 512):
                                if rb == 3:
                                    while gen is not None:
                                        gen = pull(gen, 4)
                                    pump(len(pending))
                                rsl = slice(rb * 512, (rb + 1) * 512)
                                if rb == 2:
                                    pass  # drain point moved into oc loop below
                                for oc in range(NCC):
                                    pso = ops2.tile([128, 512], F32, tag="pso")
                                    for h in range(LH):
                                        nc.tensor.matmul(
                                            pso[:],
                                            wo_s[:, h, oc * 128:(oc + 1) * 128],
                                            y_s[:, h, rsl],
                                            start=(h == 0),
                                            stop=(h == LH - 1),
                                        )
                                    ot = ooutp.tile([128, 512], F32, tag="ot")
                                    if ocount % 2 == 0:
                                        nc.scalar.copy(ot[:], pso[:])
                                    else:
                                        nc.vector.tensor_copy(ot[:], pso[:])
                                    nc.gpsimd.dma_start(
                                        out=outT.ap()[oc * 128:(oc + 1) * 128, rsl],
                                        in_=ot[:],
                                    )
                                    ocount += 1
                                    gen = pull(gen, 2)
                            pump(len(pending))

    nc.finalize()
    return nc


_NC_CACHE = None


def get_nc():
    global _NC_CACHE
    if _NC_CACHE is None:
        _NC_CACHE = _build_nc()
    return _NC_CACHE


def build_nrep(nrep):
    return _build_nc(nrep=nrep)


def _trig_tables(offset):
    inv_freq = 1.0 / (ROPE_BASE ** (np.arange(0, HD, 2, dtype=np.float64) / HD))
    pos = np.arange(offset, offset + T, dtype=np.float64)
    ang = pos[:, None] * inv_freq[None, :]        # [T, 64]
    cos = np.cos(ang)
    sin = np.sin(ang)
    cosT = np.concatenate([cos, cos], axis=1).T.astype(np.float32)   # [128, T]
    sinT = np.concatenate([-sin, sin], axis=1).T.astype(np.float32)  # sign-folded
    return np.ascontiguousarray(cosT), np.ascontiguousarray(sinT)


def _mask_table():
    """Diagonal-chunk patterns (dup per head of the pair):
    di=0 [128,512]: key ki vs query qi (0..255): 0 if ki <= qi;
    di=1 [128,512]: 0 if 128+ki <= qi (kept for reference);
    half [128,256]: di=1 restricted to qi in 128..255 -> 0 if ki <= qi-128."""
    ki = np.arange(128)
    qi = np.arange(256)
    m = np.zeros((128, 768), dtype=np.float32)
    b0 = np.where(ki[:, None] <= qi[None, :], 0.0, MASK_NEG).astype(np.float32)
    m[:, 0:256] = b0
    m[:, 256:512] = b0
    tri = b0[:, 0:128]
    m[:, 512:640] = tri
    m[:, 640:768] = tri
    return m


def make_in_maps(x, Wq, Wk, Wv, Wo, offset):
    bf16 = ml_dtypes.bfloat16
    x = np.asarray(x, dtype=np.float32)
    Wq = np.asarray(Wq, dtype=np.float32)
    Wk = np.asarray(Wk, dtype=np.float32)
    Wv = np.asarray(Wv, dtype=np.float32)
    Wo = np.asarray(Wo, dtype=np.float32)
    offset = int(np.asarray(offset))

    scale = 1.0 / math.sqrt(HD)
    cosT, sinT = _trig_tables(offset)
    mask = _mask_table()
    ones = np.ones((128, 128), dtype=np.float32)

    in_maps = []
    for core in range(8):
        b, h = core // 2, core % 2
        qrows = slice(h * LH * HD, (h + 1) * LH * HD)
        kvrows = slice(h * LG * HD, (h + 1) * LG * HD)
        in_maps.append({
            "xT": np.ascontiguousarray(x[b].T).astype(bf16),
            "wqT": np.ascontiguousarray((Wq[qrows] * scale).T).astype(bf16),
            "wkT": np.ascontiguousarray(Wk[kvrows].T).astype(bf16),
            "wvT": np.ascontiguousarray(Wv[kvrows].T).astype(bf16),
            "woT": np.ascontiguousarray(Wo[:, qrows].T).astype(bf16),
            "cosk": cosT.astype(bf16), "sink": sinT.astype(bf16),
            "maskadd": mask.astype(bf16),
            "ones_d": ones,
        })
    return in_maps


def assemble_output(results):
    out = np.empty((B, T, C), dtype=np.float32)
    for b in range(B):
        p0 = results[2 * b]["outT"]
        p1 = results[2 * b + 1]["outT"]
        out[b] = (p0 + p1).T
    return out


def kernel(x, Wq, Wk, Wv, Wo, offset):
    nc = get_nc()
    in_maps = make_in_maps(x, Wq, Wk, Wv, Wo, offset)
    res = run_bass_kernel_spmd(nc, in_maps, core_ids=list(range(8)))
    return assemble_output(res.results)

